# revision 33
# baseline (speedup 1.0000x reference)
"""MultiBoxLoss (SSD) Bass/Trainium2 kernel.

Data-parallel over batch: 64 rows -> 8 cores x 8 rows. Each core computes
partial sums [loss_l_num, sum_pos_lse, sum_pos_conf_gt, S_neg_total, num_pos]
fully on device; host combines 8x5 scalars and divides by N.

Prior layout on chip: prior i -> (partition p = i // 196, free f = i % 196),
padded to 128*196 = 25088 (88 pad priors). Mining uses a repacked layout
[128 = 8 rows x 16 chunks, 1568] via a DRAM round trip, with a fully
tensorized 28-iteration bisection for the top-k threshold per row.
"""

from contextlib import ExitStack

import numpy as np

import concourse.bass as bass
import concourse.bacc as bacc
import concourse.tile as tile
from concourse import mybir
from concourse import bass_utils

F32 = mybir.dt.float32
I32 = mybir.dt.int32
U8 = mybir.dt.uint8
OP = mybir.AluOpType
AF = mybir.ActivationFunctionType

B, P, T, C = 64, 25000, 16, 81
NCORES = 8
R = B // NCORES          # rows per core
NP = 196                 # priors per partition
PADP = 128 * NP          # 25088
NPAD = PADP - P          # 88
FULLP = P // NP          # 127 full partitions
TAILF = P - FULLP * NP   # 108 valid f on partition 127
GC = 28                  # conf chunk f-width (7 chunks of 28 = 196)
NCH = NP // GC           # 7
MCH = PADP // 16         # 1568 mining chunk width; 8 rows * 16 chunks = 128
N_ITERS = 28


def build_program():
    nc = bacc.Bacc("TRN2", target_bir_lowering=False, debug=False)
    loc = nc.dram_tensor("loc", [R, P, 4], F32, kind="ExternalInput").ap()
    conf = nc.dram_tensor("conf", [R, P, C], F32, kind="ExternalInput").ap()
    priors = nc.dram_tensor("priors", [P, 4], F32, kind="ExternalInput").ap()
    targets = nc.dram_tensor("targets", [R, T, 5], F32, kind="ExternalInput").ap()
    out = nc.dram_tensor("out", [1, 8], F32, kind="ExternalOutput").ap()
    scratch = nc.dram_tensor("scratch", [R, PADP], F32, kind="Internal").ap()
    sb16 = nc.dram_tensor("sb16", [R, 16], F32, kind="Internal").ap()
    sk8 = nc.dram_tensor("sk8", [1, 8], F32, kind="Internal").ap()

    with TileKernel(nc) as tk:
        tk.sb16 = sb16
        tk.sk8 = sk8
        tk.build(loc, conf, priors, targets, out, scratch)
    nc.compile()
    return nc


class TileKernel:
    def __init__(self, nc):
        self.nc = nc
        self.ctx = ExitStack()
        self.tc = None

    def __enter__(self):
        self.tc = self.ctx.enter_context(tile.TileContext(self.nc))
        return self

    def __exit__(self, *a):
        return self.ctx.__exit__(*a)

    def build(self, loc, conf, priors, targets, out, scratch):
        nc = self.nc
        tc = self.tc
        ctx = self.ctx
        consts = ctx.enter_context(tc.tile_pool(name="consts", bufs=1))
        work = ctx.enter_context(tc.tile_pool(name="work", bufs=2))
        big3 = ctx.enter_context(tc.tile_pool(name="big3", bufs=1))
        small = ctx.enter_context(tc.tile_pool(name="small", bufs=2))
        acc = ctx.enter_context(tc.tile_pool(name="acc", bufs=1))
        psum = ctx.enter_context(tc.tile_pool(name="psum", bufs=8, space="PSUM"))
        self.big3 = big3

        # ---------------- constants ----------------
        iop_i = consts.tile([128, 1], I32, tag="c0")
        nc.gpsimd.iota(iop_i, pattern=[[0, 1]], base=0, channel_multiplier=1)
        iom128_i = consts.tile([128, 128], I32, tag="c1")
        nc.gpsimd.iota(iom128_i, pattern=[[1, 128]], base=0, channel_multiplier=0)
        lin_i = consts.tile([128, NP], I32, tag="c2")
        nc.gpsimd.iota(lin_i, pattern=[[1, NP]], base=0, channel_multiplier=NP)
        lin81_i = consts.tile([128, NP], I32, tag="c3")
        nc.gpsimd.iota(lin81_i, pattern=[[C, NP]], base=0, channel_multiplier=NP * C)
        io16_i = consts.tile([128, 16], I32, tag="c4")
        nc.gpsimd.iota(io16_i, pattern=[[1, 16]], base=0, channel_multiplier=0)
        iof_i = consts.tile([128, NP], I32, tag="c5")
        nc.gpsimd.iota(iof_i, pattern=[[1, NP]], base=0, channel_multiplier=0)
        iom8_i = consts.tile([128, 8], I32, tag="c6")
        nc.gpsimd.iota(iom8_i, pattern=[[1, 8]], base=0, channel_multiplier=0)

        # group id p//16 and derived selectors (compares done in f32)
        gp_i = consts.tile([128, 1], I32, tag="c7")
        nc.vector.tensor_scalar(gp_i, iop_i, 4, None, OP.arith_shift_right)
        gp_f = consts.tile([128, 1], F32, tag="c7f")
        nc.vector.tensor_copy(gp_f, gp_i)
        iop_f = consts.tile([128, 1], F32, tag="c0f")
        nc.vector.tensor_copy(iop_f, iop_i)
        m16_i = consts.tile([128, 128], I32, tag="c8")
        nc.vector.tensor_scalar(m16_i, iom128_i, 4, None, OP.arith_shift_right)
        m16_f = consts.tile([128, 128], F32, tag="c8f")
        nc.vector.tensor_copy(m16_f, m16_i)
        G128 = consts.tile([128, 128], F32, tag="c9")
        nc.vector.tensor_scalar(G128, m16_f, gp_f[:, 0:1], None, OP.is_equal)
        iom8_f = consts.tile([128, 8], F32, tag="c6f")
        nc.vector.tensor_copy(iom8_f, iom8_i)
        sel8 = consts.tile([128, 8], F32, tag="c11")
        nc.vector.tensor_scalar(sel8, iom8_f, gp_f[:, 0:1], None, OP.is_equal)
        and_i = consts.tile([128, 1], I32, tag="c12")
        nc.vector.tensor_scalar(and_i, iop_i, 15, None, OP.bitwise_and)
        selone = consts.tile([128, 1], F32, tag="c13")
        nc.vector.tensor_copy(selone, and_i)
        nc.vector.tensor_scalar(selone, selone, 0.0, None, OP.is_equal)

        # float iotas
        io16f = consts.tile([128, 16], F32, tag="c14")
        nc.vector.tensor_copy(io16f, io16_i)
        rev16 = consts.tile([128, 16], F32, tag="c15")
        nc.vector.tensor_scalar(rev16, io16f, -1.0, 16.0, OP.mult, OP.add)
        jp1 = consts.tile([128, 16], F32, tag="c16")
        nc.vector.tensor_scalar(jp1, io16f, 1.0, None, OP.add)
        ioff = consts.tile([128, NP], F32, tag="c17")
        nc.vector.tensor_copy(ioff, iof_i)
        revf = consts.tile([128, NP], F32, tag="c18")
        nc.vector.tensor_scalar(revf, ioff, -1.0, float(NP), OP.mult, OP.add)
        iomf = consts.tile([128, 128], F32, tag="c19")
        nc.vector.tensor_copy(iomf, iom128_i)
        revp = consts.tile([128, 128], F32, tag="c20")
        nc.vector.tensor_scalar(revp, iomf, -1.0, 128.0, OP.mult, OP.add)
        ident = consts.tile([128, 128], F32, tag="c10")
        nc.vector.tensor_scalar(ident, iomf, iop_f[:, 0:1], None, OP.is_equal)
        linf = consts.tile([128, NP], F32, tag="c21")
        nc.vector.tensor_copy(linf, lin_i)
        lin81f = consts.tile([128, NP], F32, tag="c22")
        nc.vector.tensor_copy(lin81f, lin81_i)
        validm = consts.tile([128, NP], F32, tag="c23")
        nc.vector.tensor_scalar(validm, linf, float(P), None, OP.is_lt)
        ones_1x128 = consts.tile([1, 128], F32, tag="c24")
        nc.vector.memset(ones_1x128, 1.0)
        ones_128x1 = consts.tile([128, 1], F32, tag="c25")
        nc.vector.memset(ones_128x1, 1.0)
        two_c = consts.tile([128, 1], F32, tag="c26")
        nc.vector.memset(two_c, 2.0)

        # ---------------- priors (shared across rows) ----------------
        pri = consts.tile([128, NP, 4], F32, tag="pri")
        nc.vector.memset(pri[96:128, TAILF:NP, :], 1e-3)
        nc.vector.memset(pri[96:128, TAILF:NP, 0:2], -100.0)
        nc.sync.dma_start(
            out=pri[:FULLP, :, :],
            in_=priors[: FULLP * NP, :].rearrange("(p f) c -> p f c", f=NP),
        )
        nc.sync.dma_start(
            out=pri[FULLP : FULLP + 1, :TAILF, :],
            in_=priors[FULLP * NP : P, :].rearrange("(p f) c -> p f c", p=1),
        )
        pcx, pcy, pw, ph = (pri[:, :, i] for i in range(4))
        px0 = consts.tile([128, NP], F32, tag="px0")
        nc.vector.scalar_tensor_tensor(px0, pw, -0.5, pcx, OP.mult, OP.add)
        py0 = consts.tile([128, NP], F32, tag="py0")
        nc.vector.scalar_tensor_tensor(py0, ph, -0.5, pcy, OP.mult, OP.add)
        px1 = consts.tile([128, NP], F32, tag="px1")
        nc.vector.scalar_tensor_tensor(px1, pw, 0.5, pcx, OP.mult, OP.add)
        py1 = consts.tile([128, NP], F32, tag="py1")
        nc.vector.scalar_tensor_tensor(py1, ph, 0.5, pcy, OP.mult, OP.add)
        parea = consts.tile([128, NP], F32, tag="parea")
        nc.vector.tensor_tensor(parea, pw, ph, OP.mult)
        rw10 = consts.tile([128, NP], F32, tag="rw10")
        nc.vector.reciprocal(rw10, pw)
        nc.vector.tensor_scalar(rw10, rw10, 10.0, None, OP.mult)
        rh10 = consts.tile([128, NP], F32, tag="rh10")
        nc.vector.reciprocal(rh10, ph)
        nc.vector.tensor_scalar(rh10, rh10, 10.0, None, OP.mult)
        lnpw = consts.tile([128, NP], F32, tag="lnpw")
        nc.scalar.activation(lnpw, pw, AF.Ln)
        lnph = consts.tile([128, NP], F32, tag="lnph")
        nc.scalar.activation(lnph, ph, AF.Ln)

        # ---------------- accumulators ----------------
        llacc = acc.tile([128, 1], F32, tag="llacc")
        nc.vector.memset(llacc, 0.0)
        blacc = acc.tile([128, 1], F32, tag="blacc")
        nc.vector.memset(blacc, 0.0)
        bcacc = acc.tile([128, 1], F32, tag="bcacc")
        nc.vector.memset(bcacc, 0.0)
        npmat = acc.tile([128, 8], F32, tag="npmat")

        # ---------------- per-row pipeline ----------------
        for r in range(R):
            self.row(
                r, loc, conf, targets, scratch, work, small, psum,
                dict(
                    px0=px0, py0=py0, px1=px1, py1=py1, parea=parea,
                    rw10=rw10, rh10=rh10, lnpw=lnpw, lnph=lnph,
                    rev16=rev16, jp1=jp1, revf=revf, revp=revp, iomf=iomf,
                    io16f=io16f, linf=linf, lin81f=lin81f, validm=validm,
                    ones_1x128=ones_1x128, ones_128x1=ones_128x1,
                    ident=ident, two_c=two_c, pcx=pcx, pcy=pcy,
                ),
                llacc, blacc, bcacc, npmat,
            )

        # ---------------- mining ----------------
        self.mining(scratch, out, work, small, psum, acc,
                    G128, sel8, selone, ident, ones_1x128, ones_128x1,
                    llacc, blacc, bcacc, npmat)

    # ------------------------------------------------------------------
    def row(self, r, loc, conf, targets, scratch, work, small, psum, cst,
            llacc, blacc, bcacc, npmat):
        nc = self.nc
        b3 = lambda ap: ap[:, :, None].to_broadcast([128, NP, 16])
        t3 = lambda ap: ap[:, None, :].to_broadcast([128, NP, 16])

        # --- truths broadcast to all partitions via stride-0 DMA ---
        tgt = work.tile([128, T, 5], F32, tag="tgt")
        tr = targets[r]
        nc.sync.dma_start(
            out=tgt,
            in_=bass.AP(tensor=tr.tensor, offset=tr.offset,
                        ap=[[0, 128]] + list(tr.ap)))
        tx0, ty0, tx1, ty1, tlab = (tgt[:, :, i] for i in range(5))
        tarea = work.tile([128, 16], F32, tag="tarea")
        twx = work.tile([128, 16], F32, tag="twx")
        nc.vector.tensor_tensor(twx, tx1, tx0, OP.subtract)
        nc.vector.tensor_tensor(tarea, ty1, ty0, OP.subtract)
        nc.vector.tensor_tensor(tarea, tarea, twx, OP.mult)

        # --- IoU [128, NP, 16] ---
        iou = self.big3.tile([128, NP, 16], F32, tag="iou")
        s3 = self.big3.tile([128, NP, 16], F32, tag="s3")   # scratch 3d
        s3b = self.big3.tile([128, NP, 16], F32, tag="s3b")
        # inter_x in s3, inter_y in s3b, inter in iou
        nc.vector.tensor_tensor(s3, b3(cst["px1"]), t3(tx1), OP.min)
        nc.vector.tensor_tensor(iou, b3(cst["px0"]), t3(tx0), OP.max)
        nc.vector.tensor_tensor(s3, s3, iou, OP.subtract)
        nc.scalar.activation(s3, s3, AF.Relu)
        nc.vector.tensor_tensor(s3b, b3(cst["py1"]), t3(ty1), OP.min)
        nc.vector.tensor_tensor(iou, b3(cst["py0"]), t3(ty0), OP.max)
        nc.vector.tensor_tensor(s3b, s3b, iou, OP.subtract)
        nc.scalar.activation(s3b, s3b, AF.Relu)
        nc.vector.tensor_tensor(s3, s3, s3b, OP.mult)          # inter
        nc.vector.tensor_tensor(s3b, b3(cst["parea"]), t3(tarea), OP.add)
        nc.vector.scalar_tensor_tensor(s3b, s3, -1.0, s3b, OP.mult, OP.add)  # union
        nc.vector.reciprocal(s3b, s3b)
        nc.vector.tensor_tensor(iou, s3, s3b, OP.mult)

        # --- per-prior best truth ---
        bto = work.tile([128, NP], F32, tag="bto")
        nc.vector.tensor_reduce(bto, iou, mybir.AxisListType.X, OP.max)
        nc.vector.tensor_tensor(s3, iou, b3(bto), OP.is_ge)
        nc.vector.tensor_tensor(s3, s3, t3(cst["rev16"]), OP.mult)
        bti = work.tile([128, NP], F32, tag="bti")
        nc.vector.tensor_reduce(bti, s3, mybir.AxisListType.X, OP.max)
        nc.vector.tensor_scalar(bti, bti, -1.0, 16.0, OP.mult, OP.add)

        # --- best prior per truth ---
        iou_t = iou[:].rearrange("p f t -> p t f")
        pmax = work.tile([128, 16], F32, tag="pmax")
        nc.vector.tensor_reduce(pmax, iou_t, mybir.AxisListType.X, OP.max)
        s3t = self.big3.tile([128, 16, NP], F32, tag="s3t")
        nc.vector.tensor_tensor(
            s3t, iou_t, pmax[:, :, None].to_broadcast([128, 16, NP]), OP.is_ge)
        nc.vector.tensor_tensor(
            s3t, s3t,
            cst["revf"][:, None, :].to_broadcast([128, 16, NP]), OP.mult)
        f1 = work.tile([128, 16], F32, tag="f1")
        nc.vector.tensor_reduce(f1, s3t, mybir.AxisListType.X, OP.max)
        nc.vector.tensor_scalar(f1, f1, -1.0, float(NP), OP.mult, OP.add)
        # transpose pmax, f1 -> [16,128] (both at base partition 0)
        tp_ps = psum.tile([16, 128], F32, tag="ps")
        nc.tensor.transpose(tp_ps, pmax, cst["ident"])
        pmaxT = small.tile([16, 128], F32, tag="pmaxT")
        nc.scalar.copy(pmaxT, tp_ps)
        tp2_ps = psum.tile([16, 128], F32, tag="ps")
        nc.tensor.transpose(tp2_ps, f1, cst["ident"])
        f1T = small.tile([16, 128], F32, tag="f1T")
        nc.scalar.copy(f1T, tp2_ps)
        gmax = small.tile([16, 1], F32, tag="gmax")
        nc.vector.tensor_reduce(gmax, pmaxT, mybir.AxisListType.X, OP.max)
        m2 = small.tile([16, 128], F32, tag="m2")
        nc.vector.tensor_scalar(m2, pmaxT, gmax[:, 0:1], None, OP.is_ge)
        nc.vector.tensor_tensor(m2, m2, cst["revp"][:16, :], OP.mult)
        p1 = small.tile([16, 1], F32, tag="p1")
        nc.vector.tensor_reduce(p1, m2, mybir.AxisListType.X, OP.max)
        nc.vector.tensor_scalar(p1, p1, -1.0, 128.0, OP.mult, OP.add)
        oh = small.tile([16, 128], F32, tag="oh")
        nc.vector.tensor_scalar(oh, cst["iomf"][:16, :], p1[:, 0:1], None, OP.is_equal)
        fsel = small.tile([16, 1], F32, tag="fsel")
        ohs = small.tile([16, 128], F32, tag="ohs")
        nc.vector.scalar_tensor_tensor(ohs, f1T, 1.0, oh, OP.mult, OP.mult,
                                       accum_out=fsel)
        bpi = small.tile([16, 1], F32, tag="bpi")
        nc.vector.scalar_tensor_tensor(bpi, p1, float(NP), fsel, OP.mult, OP.add)
        # broadcast bpi -> [128, 16] via DRAM bounce + stride-0 DMA
        sbr = self.sb16[r]
        nc.sync.dma_start(out=sbr.rearrange("t -> t ()"), in_=bpi)
        bpiB = work.tile([128, 16], F32, tag="bpiB")
        nc.sync.dma_start(
            out=bpiB,
            in_=bass.AP(tensor=sbr.tensor, offset=sbr.offset,
                        ap=[[0, 128]] + list(sbr.ap)))

        # --- override forced priors ---
        nc.vector.tensor_tensor(s3, b3(cst["linf"]), t3(bpiB[:, :]), OP.is_equal)
        nc.vector.tensor_tensor(s3, s3, t3(cst["jp1"]), OP.mult)
        ovr = work.tile([128, NP], F32, tag="ovr")
        nc.vector.tensor_reduce(ovr, s3, mybir.AxisListType.X, OP.max)
        ovp = work.tile([128, NP], U8, tag="ovp")
        nc.vector.tensor_scalar(ovp, ovr, 1.0, None, OP.is_ge)
        nc.vector.copy_predicated(bto, ovp, cst["two_c"][:, 0:1].to_broadcast([128, NP]))
        nc.vector.tensor_scalar(ovr, ovr, -1.0, None, OP.add)
        nc.vector.copy_predicated(bti, ovp, ovr)

        # --- pos / conf_t ---
        pos = work.tile([128, NP], F32, tag="pos")
        nc.vector.tensor_scalar(pos, bto, 0.5, None, OP.is_ge)
        nc.vector.tensor_tensor(pos, pos, cst["validm"], OP.mult)
        # mask3 = onehot(bti) over truth axis
        nc.vector.tensor_tensor(s3, t3(cst["io16f"]), b3(bti), OP.is_equal)
        labg = work.tile([128, NP], F32, tag="labg")
        nc.vector.tensor_tensor(s3b, s3, t3(tlab), OP.mult)
        nc.vector.tensor_reduce(labg, s3b, mybir.AxisListType.X, OP.add)
        conf_t = work.tile([128, NP], F32, tag="conf_t")
        nc.vector.scalar_tensor_tensor(conf_t, labg, 1.0, pos, OP.add, OP.mult)

        # --- localization loss ---
        lt = work.tile([128, NP, 4], F32, tag="lt")
        nc.vector.memset(lt[96:128, TAILF:, :], 0.0)
        nc.sync.dma_start(
            out=lt[:FULLP, :, :],
            in_=loc[r, : FULLP * NP, :].rearrange("(p f) c -> p f c", f=NP))
        nc.sync.dma_start(
            out=lt[FULLP : FULLP + 1, :TAILF, :],
            in_=loc[r, FULLP * NP : P, :].rearrange("(p f) c -> p f c", p=1))
        mc = [work.tile([128, NP], F32, tag=f"mc{i}", name=f"mc{i}")
              for i in range(4)]
        for i, tc_ in enumerate((tx0, ty0, tx1, ty1)):
            nc.vector.tensor_tensor(s3b, s3, t3(tc_), OP.mult)
            nc.vector.tensor_reduce(mc[i], s3b, mybir.AxisListType.X, OP.add)
        sl = work.tile([128, NP], F32, tag="sl")     # accumulated smooth l1
        g = work.tile([128, NP], F32, tag="g")
        tmp = work.tile([128, NP], F32, tag="tmp")
        tmq = work.tile([128, NP], F32, tag="tmq")
        msl = work.tile([128, NP], F32, tag="msl")
        mslu = work.tile([128, NP], U8, tag="mslu")
        for i in range(4):
            if i < 2:
                ctr, rr = (cst["pcx"], cst["rw10"]) if i == 0 else (cst["pcy"], cst["rh10"])
                nc.vector.tensor_tensor(g, mc[i], mc[i + 2], OP.add)
                nc.vector.scalar_tensor_tensor(g, g, 0.5, ctr, OP.mult, OP.subtract)
                nc.vector.tensor_tensor(g, g, rr, OP.mult)
            else:
                lnp = cst["lnpw"] if i == 2 else cst["lnph"]
                nc.vector.tensor_tensor(g, mc[i], mc[i - 2], OP.subtract)
                nc.scalar.activation(g, g, AF.Ln)
                nc.vector.scalar_tensor_tensor(g, lnp, -1.0, g, OP.mult, OP.add)
                nc.vector.tensor_scalar(g, g, 5.0, None, OP.mult)
            nc.vector.tensor_tensor(tmp, lt[:, :, i], g, OP.subtract)
            nc.scalar.activation(tmp, tmp, AF.Abs)
            nc.scalar.activation(tmq, tmp, AF.Square, scale=0.7071067811865476)
            nc.vector.tensor_scalar(mslu, tmp, 1.0, None, OP.is_lt)
            nc.vector.tensor_scalar(tmp, tmp, -0.5, None, OP.add)
            nc.vector.copy_predicated(tmp, mslu, tmq)
            if i == 0:
                nc.vector.tensor_copy(sl, tmp)
            else:
                nc.vector.tensor_tensor(sl, sl, tmp, OP.add)
        llrow = small.tile([128, 1], F32, tag="llrow")
        nc.vector.scalar_tensor_tensor(msl, sl, 1.0, pos, OP.mult, OP.mult,
                                       accum_out=llrow)
        nc.vector.tensor_tensor(llacc, llacc, llrow, OP.add)

        # --- confidence: lse, conf0 ---
        serow = work.tile([128, NP], F32, tag="serow")
        c0row = work.tile([128, NP], F32, tag="c0row")
        for ch in range(NCH):
            f0 = ch * GC
            lastf = max(0, min(GC, TAILF - f0))
            cf = work.tile([128, GC, C], F32, tag="cf")
            et = self.big3.tile([128, GC, C], F32, tag="et")
            if lastf < GC:
                nc.vector.memset(cf[96:128, :, :], 0.0)
            nc.sync.dma_start(
                out=cf[:FULLP],
                in_=conf[r, : FULLP * NP, :]
                .rearrange("(p f) c -> p f c", f=NP)[:, f0 : f0 + GC, :])
            if lastf > 0:
                nc.sync.dma_start(
                    out=cf[FULLP : FULLP + 1, :lastf, :],
                    in_=conf[r, FULLP * NP + f0 : FULLP * NP + f0 + lastf, :]
                    .rearrange("(p f) c -> p f c", p=1))
            nc.scalar.activation(et, cf, AF.Exp)
            nc.vector.tensor_reduce(serow[:, f0 : f0 + GC], et,
                                    mybir.AxisListType.X, OP.add)
            nc.vector.tensor_copy(c0row[:, f0 : f0 + GC], cf[:, :, 0])
        lse = work.tile([128, NP], F32, tag="lse")
        nc.scalar.activation(lse, serow, AF.Ln)
        blrow = small.tile([128, 1], F32, tag="blrow")
        nc.vector.scalar_tensor_tensor(serow, lse, 1.0, pos, OP.mult, OP.mult,
                                       accum_out=blrow)
        nc.vector.tensor_tensor(blacc, blacc, blrow, OP.add)

        # --- conf_gt gather + masked sum ---
        offf = work.tile([128, NP], F32, tag="offf")
        nc.vector.tensor_tensor(offf, cst["lin81f"], conf_t, OP.add)
        nc.vector.tensor_tensor(offf, offf, cst["validm"], OP.mult)
        offi = work.tile([128, NP], I32, tag="offi")
        nc.vector.tensor_copy(offi, offf)
        cg = work.tile([128, NP], F32, tag="cg")
        conf_flat = conf.rearrange("r p c -> (r p) c")
        nc.gpsimd.indirect_dma_start(
            out=cg, out_offset=None, in_=conf_flat,
            in_offset=bass.IndirectOffsetOnAxis(ap=offi, axis=1),
            element_offset=r * P * C)
        bcrow = small.tile([128, 1], F32, tag="bcrow")
        nc.vector.scalar_tensor_tensor(cg, cg, 1.0, pos, OP.mult, OP.mult,
                                       accum_out=bcrow)
        nc.vector.tensor_tensor(bcacc, bcacc, bcrow, OP.add)

        # --- num_pos, loss_c -> scratch ---
        nprow = npmat[:, r : r + 1]
        nc.vector.tensor_reduce(nprow, pos, mybir.AxisListType.X, OP.add)
        lc = work.tile([128, NP], F32, tag="lc")
        nc.vector.tensor_scalar(tmp, pos, -1.0, 1.0, OP.mult, OP.add)
        nc.vector.tensor_tensor(lc, lse, c0row, OP.subtract)
        nc.vector.tensor_tensor(lc, lc, tmp, OP.mult)
        # pad priors -> -1 (never mined): lc = (lc+1)*valid - 1
        nc.vector.tensor_scalar(lc, lc, 1.0, None, OP.add)
        nc.vector.tensor_tensor(lc, lc, cst["validm"], OP.mult)
        nc.vector.tensor_scalar(lc, lc, -1.0, None, OP.add)
        nc.sync.dma_start(
            out=scratch[r].rearrange("(p f) -> p f", f=NP), in_=lc)

    # ------------------------------------------------------------------
    def mining(self, scratch, out, work, small, psum, acc,
               G128, sel8, selone, ident, ones_1x128, ones_128x1,
               llacc, blacc, bcacc, npmat):
        nc = self.nc
        # per-row num_pos totals: [8,1] = npmat^T @ ones
        np_ps = psum.tile([8, 1], F32, tag="ps")
        nc.tensor.matmul(np_ps, npmat, ones_128x1, start=True, stop=True)
        npv = small.tile([8, 1], F32, tag="npv")
        nc.scalar.copy(npv, np_ps)
        # N total
        e_ps = psum.tile([1, 1], F32, tag="ps")
        nc.tensor.matmul(e_ps, npv, ones_128x1[:8, :], start=True, stop=True)
        # k per row
        kv = small.tile([8, 1], F32, tag="kv")
        nc.vector.tensor_scalar(kv, npv, 3.0, float(P - 1), OP.mult, OP.min)
        nc.sync.dma_start(out=self.sk8.rearrange("o e -> (o e) ()"), in_=kv)
        kb = small.tile([128, 8], F32, tag="kb")
        nc.sync.dma_start(
            out=kb,
            in_=bass.AP(tensor=self.sk8.tensor, offset=0,
                        ap=[[0, 128], [1, 8]]))
        k128 = small.tile([128, 1], F32, tag="k128")
        ks = small.tile([128, 8], F32, tag="ks")
        nc.vector.scalar_tensor_tensor(ks, kb, 1.0, sel8, OP.mult, OP.mult,
                                       accum_out=k128)

        # loss_c packed [128, 1568]
        lcp = acc.tile([128, MCH], F32, tag="lcp")
        nc.sync.dma_start(
            out=lcp,
            in_=bass.AP(tensor=scratch.tensor, offset=0,
                        ap=[[MCH, 128], [1, MCH]]))

        lo = small.tile([128, 1], F32, tag="lo")
        nc.vector.memset(lo, 0.0)
        hi = small.tile([128, 1], F32, tag="hi")
        nc.vector.memset(hi, 12.0)
        mid = small.tile([128, 1], F32, tag="mid")
        msk = acc.tile([128, MCH], F32, tag="msk")
        for it in range(N_ITERS):
            nc.vector.tensor_tensor(mid, lo, hi, OP.add)
            nc.scalar.mul(mid, mid, 0.5)
            pc = small.tile([128, 1], F32, tag="pc")
            nc.vector.tensor_scalar(msk, lcp, mid[:, 0:1], None, OP.is_gt,
                                    OP.add, accum_out=pc)
            c_ps = psum.tile([128, 1], F32, tag="ps")
            nc.tensor.matmul(c_ps, G128, pc, start=True, stop=True)
            cntf = small.tile([128, 1], F32, tag="cntf")
            nc.scalar.copy(cntf, c_ps)
            sel = small.tile([128, 1], U8, tag="sel")
            nc.vector.tensor_scalar(sel, cntf, k128[:, 0:1], None, OP.is_ge)
            nc.vector.copy_predicated(lo, sel, mid)
            sel2 = small.tile([128, 1], U8, tag="sel2")
            nc.vector.tensor_scalar(sel2, cntf, k128[:, 0:1], None, OP.is_lt)
            nc.vector.copy_predicated(hi, sel2, mid)

        # final masked sum + count at threshold lo
        st2 = small.tile([128, 2], F32, tag="st2")
        nc.vector.scalar_tensor_tensor(msk, lcp, lo[:, 0:1], lcp, OP.is_gt,
                                       OP.mult, accum_out=st2[:, 0:1])
        nc.vector.tensor_scalar(msk, lcp, lo[:, 0:1], None, OP.is_gt,
                                OP.add, accum_out=st2[:, 1:2])
        g2_ps = psum.tile([128, 2], F32, tag="ps")
        nc.tensor.matmul(g2_ps, G128, st2, start=True, stop=True)
        gt2 = small.tile([128, 2], F32, tag="gt2")
        nc.scalar.copy(gt2, g2_ps)
        sn = small.tile([128, 1], F32, tag="sn")
        nc.vector.tensor_tensor(sn, gt2[:, 1:2], k128, OP.subtract)
        nc.vector.tensor_tensor(sn, sn, lo, OP.mult)
        nc.vector.tensor_tensor(sn, gt2[:, 0:1], sn, OP.subtract)
        d_ps = psum.tile([1, 1], F32, tag="ps")
        nc.tensor.matmul(d_ps, sn, selone, start=True, stop=True)

        # final scalars A..E
        a_ps = psum.tile([1, 1], F32, tag="ps")
        nc.tensor.matmul(a_ps, llacc, ones_128x1, start=True, stop=True)
        b_ps = psum.tile([1, 1], F32, tag="ps")
        nc.tensor.matmul(b_ps, blacc, ones_128x1, start=True, stop=True)
        c2_ps = psum.tile([1, 1], F32, tag="ps")
        nc.tensor.matmul(c2_ps, bcacc, ones_128x1, start=True, stop=True)
        outsb = small.tile([1, 8], F32, tag="outsb")
        nc.vector.memset(outsb, 0.0)
        nc.scalar.copy(outsb[:, 0:1], a_ps)
        nc.scalar.copy(outsb[:, 1:2], b_ps)
        nc.scalar.copy(outsb[:, 2:3], c2_ps)
        nc.scalar.copy(outsb[:, 3:4], d_ps)
        nc.scalar.copy(outsb[:, 4:5], e_ps)
        nc.sync.dma_start(out=out, in_=outsb)


_CACHED = {}


def kernel(loc_data, conf_data, priors, targets):
    if "nc" not in _CACHED:
        _CACHED["nc"] = build_program()
    nc = _CACHED["nc"]
    in_maps = []
    for c in range(NCORES):
        sl = slice(c * R, (c + 1) * R)
        in_maps.append({
            "loc": np.ascontiguousarray(loc_data[sl]),
            "conf": np.ascontiguousarray(conf_data[sl]),
            "priors": np.ascontiguousarray(priors),
            "targets": np.ascontiguousarray(targets[sl]),
        })
    res = bass_utils.run_bass_kernel_spmd(nc, in_maps, core_ids=list(range(NCORES)))
    _CACHED["last_results"] = res
    A = Bs = Cs = D = E = 0.0
    for c in range(NCORES):
        o = res.results[c]["out"].reshape(-1)
        A += float(o[0]); Bs += float(o[1]); Cs += float(o[2])
        D += float(o[3]); E += float(o[4])
    N = max(E, 1.0)
    return np.array([A / N, (Bs - Cs + D) / N], dtype=np.float32)


# revision 38
# speedup vs baseline: 10518.5145x; 10518.5145x over previous
"""MultiBoxLoss (SSD) Bass/Trainium2 kernel.

Data-parallel over batch: 64 rows -> 8 cores x 8 rows. Each core computes
partial sums [loss_l_num, sum_pos_lse, sum_pos_conf_gt, S_neg_total, num_pos]
fully on device; host combines 8x5 scalars and divides by N.

Prior layout on chip: prior i -> (partition p = i // 196, free f = i % 196),
padded to 128*196 = 25088 (88 pad priors). Mining uses a repacked layout
[128 = 8 rows x 16 chunks, 1568] via a DRAM round trip, with a fully
tensorized 28-iteration bisection for the top-k threshold per row.
"""

from contextlib import ExitStack

import numpy as np

import concourse.bass as bass
import concourse.bacc as bacc
import concourse.tile as tile
from concourse import mybir
from concourse import bass_utils

F32 = mybir.dt.float32
I32 = mybir.dt.int32
U8 = mybir.dt.uint8
OP = mybir.AluOpType
AF = mybir.ActivationFunctionType

B, P, T, C = 64, 25000, 16, 81
NCORES = 8
R = B // NCORES          # rows per core
NP = 196                 # priors per partition
PADP = 128 * NP          # 25088
NPAD = PADP - P          # 88
FULLP = P // NP          # 127 full partitions
TAILF = P - FULLP * NP   # 108 valid f on partition 127
GC = 28                  # conf chunk f-width (7 chunks of 28 = 196)
NCH = NP // GC           # 7
MCH = PADP // 16         # 1568 mining chunk width; 8 rows * 16 chunks = 128
N_ITERS = 28


def build_program():
    nc = bacc.Bacc("TRN2", target_bir_lowering=False, debug=False)
    loc = nc.dram_tensor("loc", [R, P, 4], F32, kind="ExternalInput").ap()
    conf = nc.dram_tensor("conf", [R, P, C], F32, kind="ExternalInput").ap()
    priors = nc.dram_tensor("priors", [P, 4], F32, kind="ExternalInput").ap()
    targets = nc.dram_tensor("targets", [R, T, 5], F32, kind="ExternalInput").ap()
    out = nc.dram_tensor("out", [1, 8], F32, kind="ExternalOutput").ap()
    scratch = nc.dram_tensor("scratch", [R, PADP], F32, kind="Internal").ap()
    sb16 = nc.dram_tensor("sb16", [R, 16], F32, kind="Internal").ap()
    sk8 = nc.dram_tensor("sk8", [1, 8], F32, kind="Internal").ap()

    with TileKernel(nc) as tk:
        tk.sb16 = sb16
        tk.sk8 = sk8
        tk.build(loc, conf, priors, targets, out, scratch)
    nc.compile()
    return nc


class TileKernel:
    def __init__(self, nc):
        self.nc = nc
        self.ctx = ExitStack()
        self.tc = None

    def __enter__(self):
        self.tc = self.ctx.enter_context(tile.TileContext(self.nc))
        return self

    def __exit__(self, *a):
        return self.ctx.__exit__(*a)

    def build(self, loc, conf, priors, targets, out, scratch):
        nc = self.nc
        tc = self.tc
        ctx = self.ctx
        consts = ctx.enter_context(tc.tile_pool(name="consts", bufs=1))
        work = ctx.enter_context(tc.tile_pool(name="work", bufs=2))
        big3 = ctx.enter_context(tc.tile_pool(name="big3", bufs=1))
        small = ctx.enter_context(tc.tile_pool(name="small", bufs=2))
        acc = ctx.enter_context(tc.tile_pool(name="acc", bufs=1))
        psum = ctx.enter_context(tc.tile_pool(name="psum", bufs=8, space="PSUM"))
        self.big3 = big3

        # ---------------- constants ----------------
        iop_i = consts.tile([128, 1], I32, tag="c0")
        nc.gpsimd.iota(iop_i, pattern=[[0, 1]], base=0, channel_multiplier=1)
        iom128_i = consts.tile([128, 128], I32, tag="c1")
        nc.gpsimd.iota(iom128_i, pattern=[[1, 128]], base=0, channel_multiplier=0)
        lin_i = consts.tile([128, NP], I32, tag="c2")
        nc.gpsimd.iota(lin_i, pattern=[[1, NP]], base=0, channel_multiplier=NP)
        lin81_i = consts.tile([128, NP], I32, tag="c3")
        nc.gpsimd.iota(lin81_i, pattern=[[C, NP]], base=0, channel_multiplier=NP * C)
        io16_i = consts.tile([128, 16], I32, tag="c4")
        nc.gpsimd.iota(io16_i, pattern=[[1, 16]], base=0, channel_multiplier=0)
        iof_i = consts.tile([128, NP], I32, tag="c5")
        nc.gpsimd.iota(iof_i, pattern=[[1, NP]], base=0, channel_multiplier=0)
        iom8_i = consts.tile([128, 8], I32, tag="c6")
        nc.gpsimd.iota(iom8_i, pattern=[[1, 8]], base=0, channel_multiplier=0)
        io81_i = consts.tile([128, C], I32, tag="c27")
        nc.gpsimd.iota(io81_i, pattern=[[1, C]], base=0, channel_multiplier=0)
        io81f = consts.tile([128, C], F32, tag="c28")
        nc.vector.tensor_copy(io81f, io81_i)
        self.io81f = io81f

        # group id p//16 and derived selectors (compares done in f32)
        gp_i = consts.tile([128, 1], I32, tag="c7")
        nc.vector.tensor_scalar(gp_i, iop_i, 4, None, OP.arith_shift_right)
        gp_f = consts.tile([128, 1], F32, tag="c7f")
        nc.vector.tensor_copy(gp_f, gp_i)
        iop_f = consts.tile([128, 1], F32, tag="c0f")
        nc.vector.tensor_copy(iop_f, iop_i)
        m16_i = consts.tile([128, 128], I32, tag="c8")
        nc.vector.tensor_scalar(m16_i, iom128_i, 4, None, OP.arith_shift_right)
        m16_f = consts.tile([128, 128], F32, tag="c8f")
        nc.vector.tensor_copy(m16_f, m16_i)
        G128 = consts.tile([128, 128], F32, tag="c9")
        nc.vector.tensor_scalar(G128, m16_f, gp_f[:, 0:1], None, OP.is_equal)
        iom8_f = consts.tile([128, 8], F32, tag="c6f")
        nc.vector.tensor_copy(iom8_f, iom8_i)
        sel8 = consts.tile([128, 8], F32, tag="c11")
        nc.vector.tensor_scalar(sel8, iom8_f, gp_f[:, 0:1], None, OP.is_equal)
        and_i = consts.tile([128, 1], I32, tag="c12")
        nc.vector.tensor_scalar(and_i, iop_i, 15, None, OP.bitwise_and)
        selone = consts.tile([128, 1], F32, tag="c13")
        nc.vector.tensor_copy(selone, and_i)
        nc.vector.tensor_scalar(selone, selone, 0.0, None, OP.is_equal)

        # float iotas
        io16f = consts.tile([128, 16], F32, tag="c14")
        nc.vector.tensor_copy(io16f, io16_i)
        rev16 = consts.tile([128, 16], F32, tag="c15")
        nc.vector.tensor_scalar(rev16, io16f, -1.0, 16.0, OP.mult, OP.add)
        jp1 = consts.tile([128, 16], F32, tag="c16")
        nc.vector.tensor_scalar(jp1, io16f, 1.0, None, OP.add)
        ioff = consts.tile([128, NP], F32, tag="c17")
        nc.vector.tensor_copy(ioff, iof_i)
        revf = consts.tile([128, NP], F32, tag="c18")
        nc.vector.tensor_scalar(revf, ioff, -1.0, float(NP), OP.mult, OP.add)
        iomf = consts.tile([128, 128], F32, tag="c19")
        nc.vector.tensor_copy(iomf, iom128_i)
        revp = consts.tile([128, 128], F32, tag="c20")
        nc.vector.tensor_scalar(revp, iomf, -1.0, 128.0, OP.mult, OP.add)
        ident = consts.tile([128, 128], F32, tag="c10")
        nc.vector.tensor_scalar(ident, iomf, iop_f[:, 0:1], None, OP.is_equal)
        linf = consts.tile([128, NP], F32, tag="c21")
        nc.vector.tensor_copy(linf, lin_i)
        lin81f = consts.tile([128, NP], F32, tag="c22")
        nc.vector.tensor_copy(lin81f, lin81_i)
        validm = consts.tile([128, NP], F32, tag="c23")
        nc.vector.tensor_scalar(validm, linf, float(P), None, OP.is_lt)
        ones_1x128 = consts.tile([1, 128], F32, tag="c24")
        nc.vector.memset(ones_1x128, 1.0)
        ones_128x1 = consts.tile([128, 1], F32, tag="c25")
        nc.vector.memset(ones_128x1, 1.0)
        two_c = consts.tile([128, 1], F32, tag="c26")
        nc.vector.memset(two_c, 2.0)

        # ---------------- priors (shared across rows) ----------------
        pri = consts.tile([128, NP, 4], F32, tag="pri")
        nc.vector.memset(pri[96:128, TAILF:NP, :], 1e-3)
        nc.vector.memset(pri[96:128, TAILF:NP, 0:2], -100.0)
        nc.sync.dma_start(
            out=pri[:FULLP, :, :],
            in_=priors[: FULLP * NP, :].rearrange("(p f) c -> p f c", f=NP),
        )
        nc.sync.dma_start(
            out=pri[FULLP : FULLP + 1, :TAILF, :],
            in_=priors[FULLP * NP : P, :].rearrange("(p f) c -> p f c", p=1),
        )
        pcx, pcy, pw, ph = (pri[:, :, i] for i in range(4))
        px0 = consts.tile([128, NP], F32, tag="px0")
        nc.vector.scalar_tensor_tensor(px0, pw, -0.5, pcx, OP.mult, OP.add)
        py0 = consts.tile([128, NP], F32, tag="py0")
        nc.vector.scalar_tensor_tensor(py0, ph, -0.5, pcy, OP.mult, OP.add)
        px1 = consts.tile([128, NP], F32, tag="px1")
        nc.vector.scalar_tensor_tensor(px1, pw, 0.5, pcx, OP.mult, OP.add)
        py1 = consts.tile([128, NP], F32, tag="py1")
        nc.vector.scalar_tensor_tensor(py1, ph, 0.5, pcy, OP.mult, OP.add)
        parea = consts.tile([128, NP], F32, tag="parea")
        nc.vector.tensor_tensor(parea, pw, ph, OP.mult)
        rw10 = consts.tile([128, NP], F32, tag="rw10")
        nc.vector.reciprocal(rw10, pw)
        nc.vector.tensor_scalar(rw10, rw10, 10.0, None, OP.mult)
        rh10 = consts.tile([128, NP], F32, tag="rh10")
        nc.vector.reciprocal(rh10, ph)
        nc.vector.tensor_scalar(rh10, rh10, 10.0, None, OP.mult)
        lnpw = consts.tile([128, NP], F32, tag="lnpw")
        nc.scalar.activation(lnpw, pw, AF.Ln)
        lnph = consts.tile([128, NP], F32, tag="lnph")
        nc.scalar.activation(lnph, ph, AF.Ln)

        # ---------------- accumulators ----------------
        llacc = acc.tile([128, 1], F32, tag="llacc")
        nc.vector.memset(llacc, 0.0)
        blacc = acc.tile([128, 1], F32, tag="blacc")
        nc.vector.memset(blacc, 0.0)
        bcacc = acc.tile([128, 1], F32, tag="bcacc")
        nc.vector.memset(bcacc, 0.0)
        npmat = acc.tile([128, 8], F32, tag="npmat")

        # ---------------- per-row pipeline ----------------
        for r in range(R):
            self.row(
                r, loc, conf, targets, scratch, work, small, psum,
                dict(
                    px0=px0, py0=py0, px1=px1, py1=py1, parea=parea,
                    rw10=rw10, rh10=rh10, lnpw=lnpw, lnph=lnph,
                    rev16=rev16, jp1=jp1, revf=revf, revp=revp, iomf=iomf,
                    io16f=io16f, linf=linf, lin81f=lin81f, validm=validm,
                    ones_1x128=ones_1x128, ones_128x1=ones_128x1,
                    ident=ident, two_c=two_c, pcx=pcx, pcy=pcy,
                ),
                llacc, blacc, bcacc, npmat,
            )

        # ---------------- mining ----------------
        self.mining(scratch, out, work, small, psum, acc,
                    G128, sel8, selone, ident, ones_1x128, ones_128x1,
                    llacc, blacc, bcacc, npmat)

    # ------------------------------------------------------------------
    def row(self, r, loc, conf, targets, scratch, work, small, psum, cst,
            llacc, blacc, bcacc, npmat):
        nc = self.nc
        b3 = lambda ap: ap[:, :, None].to_broadcast([128, NP, 16])
        t3 = lambda ap: ap[:, None, :].to_broadcast([128, NP, 16])

        # --- truths broadcast to all partitions via stride-0 DMA ---
        tgt = work.tile([128, T, 5], F32, tag="tgt")
        tr = targets[r]
        nc.sync.dma_start(
            out=tgt,
            in_=bass.AP(tensor=tr.tensor, offset=tr.offset,
                        ap=[[0, 128]] + list(tr.ap)))
        tx0, ty0, tx1, ty1, tlab = (tgt[:, :, i] for i in range(5))
        tarea = work.tile([128, 16], F32, tag="tarea")
        twx = work.tile([128, 16], F32, tag="twx")
        nc.vector.tensor_tensor(twx, tx1, tx0, OP.subtract)
        nc.vector.tensor_tensor(tarea, ty1, ty0, OP.subtract)
        nc.vector.tensor_tensor(tarea, tarea, twx, OP.mult)

        # --- IoU [128, NP, 16] ---
        iou = self.big3.tile([128, NP, 16], F32, tag="iou")
        s3 = self.big3.tile([128, NP, 16], F32, tag="s3")   # scratch 3d
        s3b = self.big3.tile([128, NP, 16], F32, tag="s3b")
        # inter_x in s3, inter_y in s3b, inter in iou
        nc.vector.tensor_tensor(s3, b3(cst["px1"]), t3(tx1), OP.min)
        nc.vector.tensor_tensor(iou, b3(cst["px0"]), t3(tx0), OP.max)
        nc.vector.tensor_tensor(s3, s3, iou, OP.subtract)
        nc.scalar.activation(s3, s3, AF.Relu)
        nc.vector.tensor_tensor(s3b, b3(cst["py1"]), t3(ty1), OP.min)
        nc.vector.tensor_tensor(iou, b3(cst["py0"]), t3(ty0), OP.max)
        nc.vector.tensor_tensor(s3b, s3b, iou, OP.subtract)
        nc.scalar.activation(s3b, s3b, AF.Relu)
        nc.vector.tensor_tensor(s3, s3, s3b, OP.mult)          # inter
        nc.vector.tensor_tensor(s3b, b3(cst["parea"]), t3(tarea), OP.add)
        nc.vector.scalar_tensor_tensor(s3b, s3, -1.0, s3b, OP.mult, OP.add)  # union
        nc.vector.reciprocal(s3b, s3b)
        nc.vector.tensor_tensor(iou, s3, s3b, OP.mult)

        # --- per-prior best truth ---
        bto = work.tile([128, NP], F32, tag="bto")
        nc.vector.tensor_reduce(bto, iou, mybir.AxisListType.X, OP.max)
        nc.vector.tensor_tensor(s3, iou, b3(bto), OP.is_ge)
        nc.vector.tensor_tensor(s3, s3, t3(cst["rev16"]), OP.mult)
        bti = work.tile([128, NP], F32, tag="bti")
        nc.vector.tensor_reduce(bti, s3, mybir.AxisListType.X, OP.max)
        nc.vector.tensor_scalar(bti, bti, -1.0, 16.0, OP.mult, OP.add)

        # --- best prior per truth ---
        iou_t = iou[:].rearrange("p f t -> p t f")
        pmax = work.tile([128, 16], F32, tag="pmax")
        nc.vector.tensor_reduce(pmax, iou_t, mybir.AxisListType.X, OP.max)
        s3t = self.big3.tile([128, 16, NP], F32, tag="s3t")
        nc.vector.tensor_tensor(
            s3t, iou_t, pmax[:, :, None].to_broadcast([128, 16, NP]), OP.is_ge)
        nc.vector.tensor_tensor(
            s3t, s3t,
            cst["revf"][:, None, :].to_broadcast([128, 16, NP]), OP.mult)
        f1 = work.tile([128, 16], F32, tag="f1")
        nc.vector.tensor_reduce(f1, s3t, mybir.AxisListType.X, OP.max)
        nc.vector.tensor_scalar(f1, f1, -1.0, float(NP), OP.mult, OP.add)
        # transpose pmax, f1 -> [16,128] (both at base partition 0)
        tp_ps = psum.tile([16, 128], F32, tag="ps")
        nc.tensor.transpose(tp_ps, pmax, cst["ident"])
        pmaxT = small.tile([16, 128], F32, tag="pmaxT")
        nc.scalar.copy(pmaxT, tp_ps)
        tp2_ps = psum.tile([16, 128], F32, tag="ps")
        nc.tensor.transpose(tp2_ps, f1, cst["ident"])
        f1T = small.tile([16, 128], F32, tag="f1T")
        nc.scalar.copy(f1T, tp2_ps)
        gmax = small.tile([16, 1], F32, tag="gmax")
        nc.vector.tensor_reduce(gmax, pmaxT, mybir.AxisListType.X, OP.max)
        m2 = small.tile([16, 128], F32, tag="m2")
        nc.vector.tensor_scalar(m2, pmaxT, gmax[:, 0:1], None, OP.is_ge)
        nc.vector.tensor_tensor(m2, m2, cst["revp"][:16, :], OP.mult)
        p1 = small.tile([16, 1], F32, tag="p1")
        nc.vector.tensor_reduce(p1, m2, mybir.AxisListType.X, OP.max)
        nc.vector.tensor_scalar(p1, p1, -1.0, 128.0, OP.mult, OP.add)
        oh = small.tile([16, 128], F32, tag="oh")
        nc.vector.tensor_scalar(oh, cst["iomf"][:16, :], p1[:, 0:1], None, OP.is_equal)
        fsel = small.tile([16, 1], F32, tag="fsel")
        ohs = small.tile([16, 128], F32, tag="ohs")
        nc.vector.scalar_tensor_tensor(ohs, f1T, 1.0, oh, OP.mult, OP.mult,
                                       accum_out=fsel)
        bpi = small.tile([16, 1], F32, tag="bpi")
        nc.vector.scalar_tensor_tensor(bpi, p1, float(NP), fsel, OP.mult, OP.add)
        # broadcast bpi -> [128, 16] via DRAM bounce + stride-0 DMA
        sbr = self.sb16[r]
        nc.sync.dma_start(out=sbr.rearrange("t -> t ()"), in_=bpi)
        bpiB = work.tile([128, 16], F32, tag="bpiB")
        nc.sync.dma_start(
            out=bpiB,
            in_=bass.AP(tensor=sbr.tensor, offset=sbr.offset,
                        ap=[[0, 128]] + list(sbr.ap)))

        # --- override forced priors ---
        nc.vector.tensor_tensor(s3, b3(cst["linf"]), t3(bpiB[:, :]), OP.is_equal)
        nc.vector.tensor_tensor(s3, s3, t3(cst["jp1"]), OP.mult)
        ovr = work.tile([128, NP], F32, tag="ovr")
        nc.vector.tensor_reduce(ovr, s3, mybir.AxisListType.X, OP.max)
        ovp = work.tile([128, NP], U8, tag="ovp")
        nc.vector.tensor_scalar(ovp, ovr, 1.0, None, OP.is_ge)
        nc.vector.copy_predicated(bto, ovp, cst["two_c"][:, 0:1].to_broadcast([128, NP]))
        nc.vector.tensor_scalar(ovr, ovr, -1.0, None, OP.add)
        nc.vector.copy_predicated(bti, ovp, ovr)

        # --- pos / conf_t ---
        pos = work.tile([128, NP], F32, tag="pos")
        nc.vector.tensor_scalar(pos, bto, 0.5, None, OP.is_ge)
        nc.vector.tensor_tensor(pos, pos, cst["validm"], OP.mult)
        # mask3 = onehot(bti) over truth axis
        nc.vector.tensor_tensor(s3, t3(cst["io16f"]), b3(bti), OP.is_equal)
        labg = work.tile([128, NP], F32, tag="labg")
        nc.vector.tensor_tensor(s3b, s3, t3(tlab), OP.mult)
        nc.vector.tensor_reduce(labg, s3b, mybir.AxisListType.X, OP.add)
        conf_t = work.tile([128, NP], F32, tag="conf_t")
        nc.vector.scalar_tensor_tensor(conf_t, labg, 1.0, pos, OP.add, OP.mult)
        # conf_t with negatives pushed out of [0,81) so the one-hot never hits
        ct2 = work.tile([128, NP], F32, tag="ct2")
        nc.vector.tensor_scalar(ct2, conf_t, 999.0, None, OP.add)
        nc.vector.scalar_tensor_tensor(ct2, pos, -999.0, ct2, OP.mult, OP.add)

        # --- localization loss ---
        lt = work.tile([128, NP, 4], F32, tag="lt")
        nc.vector.memset(lt[96:128, TAILF:, :], 0.0)
        nc.sync.dma_start(
            out=lt[:FULLP, :, :],
            in_=loc[r, : FULLP * NP, :].rearrange("(p f) c -> p f c", f=NP))
        nc.sync.dma_start(
            out=lt[FULLP : FULLP + 1, :TAILF, :],
            in_=loc[r, FULLP * NP : P, :].rearrange("(p f) c -> p f c", p=1))
        mc = [work.tile([128, NP], F32, tag=f"mc{i}", name=f"mc{i}")
              for i in range(4)]
        for i, tc_ in enumerate((tx0, ty0, tx1, ty1)):
            nc.vector.tensor_tensor(s3b, s3, t3(tc_), OP.mult)
            nc.vector.tensor_reduce(mc[i], s3b, mybir.AxisListType.X, OP.add)
        sl = work.tile([128, NP], F32, tag="sl")     # accumulated smooth l1
        g = work.tile([128, NP], F32, tag="g")
        tmp = work.tile([128, NP], F32, tag="tmp")
        tmq = work.tile([128, NP], F32, tag="tmq")
        msl = work.tile([128, NP], F32, tag="msl")
        mslu = work.tile([128, NP], U8, tag="mslu")
        for i in range(4):
            if i < 2:
                ctr, rr = (cst["pcx"], cst["rw10"]) if i == 0 else (cst["pcy"], cst["rh10"])
                nc.vector.tensor_tensor(g, mc[i], mc[i + 2], OP.add)
                nc.vector.scalar_tensor_tensor(g, g, 0.5, ctr, OP.mult, OP.subtract)
                nc.vector.tensor_tensor(g, g, rr, OP.mult)
            else:
                lnp = cst["lnpw"] if i == 2 else cst["lnph"]
                nc.vector.tensor_tensor(g, mc[i], mc[i - 2], OP.subtract)
                nc.scalar.activation(g, g, AF.Ln)
                nc.vector.scalar_tensor_tensor(g, lnp, -1.0, g, OP.mult, OP.add)
                nc.vector.tensor_scalar(g, g, 5.0, None, OP.mult)
            nc.vector.tensor_tensor(tmp, lt[:, :, i], g, OP.subtract)
            nc.scalar.activation(tmp, tmp, AF.Abs)
            nc.scalar.activation(tmq, tmp, AF.Square, scale=0.7071067811865476)
            nc.vector.tensor_scalar(mslu, tmp, 1.0, None, OP.is_lt)
            nc.vector.tensor_scalar(tmp, tmp, -0.5, None, OP.add)
            nc.vector.copy_predicated(tmp, mslu, tmq)
            if i == 0:
                nc.vector.tensor_copy(sl, tmp)
            else:
                nc.vector.tensor_tensor(sl, sl, tmp, OP.add)
        llrow = small.tile([128, 1], F32, tag="llrow")
        nc.vector.scalar_tensor_tensor(msl, sl, 1.0, pos, OP.mult, OP.mult,
                                       accum_out=llrow)
        nc.vector.tensor_tensor(llacc, llacc, llrow, OP.add)

        # --- confidence: lse, conf0 ---
        serow = work.tile([128, NP], F32, tag="serow")
        c0row = work.tile([128, NP], F32, tag="c0row")
        for ch in range(NCH):
            f0 = ch * GC
            lastf = max(0, min(GC, TAILF - f0))
            cf = work.tile([128, GC, C], F32, tag="cf")
            et = self.big3.tile([128, GC, C], F32, tag="et")
            if lastf < GC:
                nc.vector.memset(cf[96:128, :, :], 0.0)
            nc.sync.dma_start(
                out=cf[:FULLP],
                in_=conf[r, : FULLP * NP, :]
                .rearrange("(p f) c -> p f c", f=NP)[:, f0 : f0 + GC, :])
            if lastf > 0:
                nc.sync.dma_start(
                    out=cf[FULLP : FULLP + 1, :lastf, :],
                    in_=conf[r, FULLP * NP + f0 : FULLP * NP + f0 + lastf, :]
                    .rearrange("(p f) c -> p f c", p=1))
            nc.scalar.activation(et, cf, AF.Exp)
            nc.vector.tensor_reduce(serow[:, f0 : f0 + GC], et,
                                    mybir.AxisListType.X, OP.add)
            nc.vector.tensor_copy(c0row[:, f0 : f0 + GC], cf[:, :, 0])
            # conf_gt (positives only): one-hot mask + fused mul-accum on gpsimd
            mk = self.big3.tile([128, GC, C], F32, tag="mk")
            nc.vector.tensor_tensor(
                mk, self.io81f[:, None, :].to_broadcast([128, GC, C]),
                ct2[:, f0 : f0 + GC, None].to_broadcast([128, GC, C]),
                OP.is_equal)
            bcp = small.tile([128, 1], F32, tag="bcp", name=f"bcp{ch}")
            nc.vector.scalar_tensor_tensor(mk, mk, 1.0, cf, OP.mult, OP.mult,
                                           accum_out=bcp)
            nc.vector.tensor_tensor(bcacc, bcacc, bcp, OP.add)
        lse = work.tile([128, NP], F32, tag="lse")
        nc.scalar.activation(lse, serow, AF.Ln)
        blrow = small.tile([128, 1], F32, tag="blrow")
        nc.vector.scalar_tensor_tensor(serow, lse, 1.0, pos, OP.mult, OP.mult,
                                       accum_out=blrow)
        nc.vector.tensor_tensor(blacc, blacc, blrow, OP.add)

        # --- num_pos, loss_c -> scratch ---
        nprow = npmat[:, r : r + 1]
        nc.vector.tensor_reduce(nprow, pos, mybir.AxisListType.X, OP.add)
        lc = work.tile([128, NP], F32, tag="lc")
        nc.vector.tensor_scalar(tmp, pos, -1.0, 1.0, OP.mult, OP.add)
        nc.vector.tensor_tensor(lc, lse, c0row, OP.subtract)
        nc.vector.tensor_tensor(lc, lc, tmp, OP.mult)
        # pad priors -> -1 (never mined): lc = (lc+1)*valid - 1
        nc.vector.tensor_scalar(lc, lc, 1.0, None, OP.add)
        nc.vector.tensor_tensor(lc, lc, cst["validm"], OP.mult)
        nc.vector.tensor_scalar(lc, lc, -1.0, None, OP.add)
        nc.sync.dma_start(
            out=scratch[r].rearrange("(p f) -> p f", f=NP), in_=lc)

    # ------------------------------------------------------------------
    def mining(self, scratch, out, work, small, psum, acc,
               G128, sel8, selone, ident, ones_1x128, ones_128x1,
               llacc, blacc, bcacc, npmat):
        nc = self.nc
        # per-row num_pos totals: [8,1] = npmat^T @ ones
        np_ps = psum.tile([8, 1], F32, tag="ps")
        nc.tensor.matmul(np_ps, npmat, ones_128x1, start=True, stop=True)
        npv = small.tile([8, 1], F32, tag="npv")
        nc.scalar.copy(npv, np_ps)
        # N total
        e_ps = psum.tile([1, 1], F32, tag="ps")
        nc.tensor.matmul(e_ps, npv, ones_128x1[:8, :], start=True, stop=True)
        # k per row
        kv = small.tile([8, 1], F32, tag="kv")
        nc.vector.tensor_scalar(kv, npv, 3.0, float(P - 1), OP.mult, OP.min)
        nc.sync.dma_start(out=self.sk8.rearrange("o e -> (o e) ()"), in_=kv)
        kb = small.tile([128, 8], F32, tag="kb")
        nc.sync.dma_start(
            out=kb,
            in_=bass.AP(tensor=self.sk8.tensor, offset=0,
                        ap=[[0, 128], [1, 8]]))
        k128 = small.tile([128, 1], F32, tag="k128")
        ks = small.tile([128, 8], F32, tag="ks")
        nc.vector.scalar_tensor_tensor(ks, kb, 1.0, sel8, OP.mult, OP.mult,
                                       accum_out=k128)

        # loss_c packed [128, 1568]
        lcp = acc.tile([128, MCH], F32, tag="lcp")
        nc.sync.dma_start(
            out=lcp,
            in_=bass.AP(tensor=scratch.tensor, offset=0,
                        ap=[[MCH, 128], [1, MCH]]))

        lo = small.tile([128, 1], F32, tag="lo")
        nc.vector.memset(lo, 0.0)
        hi = small.tile([128, 1], F32, tag="hi")
        nc.vector.memset(hi, 12.0)
        mid = small.tile([128, 1], F32, tag="mid")
        msk = acc.tile([128, MCH], F32, tag="msk")
        for it in range(N_ITERS):
            nc.vector.tensor_tensor(mid, lo, hi, OP.add)
            nc.scalar.mul(mid, mid, 0.5)
            pc = small.tile([128, 1], F32, tag="pc")
            nc.vector.tensor_scalar(msk, lcp, mid[:, 0:1], None, OP.is_gt,
                                    OP.add, accum_out=pc)
            c_ps = psum.tile([128, 1], F32, tag="ps")
            nc.tensor.matmul(c_ps, G128, pc, start=True, stop=True)
            cntf = small.tile([128, 1], F32, tag="cntf")
            nc.scalar.copy(cntf, c_ps)
            sel = small.tile([128, 1], U8, tag="sel")
            nc.vector.tensor_scalar(sel, cntf, k128[:, 0:1], None, OP.is_ge)
            nc.vector.copy_predicated(lo, sel, mid)
            sel2 = small.tile([128, 1], U8, tag="sel2")
            nc.vector.tensor_scalar(sel2, cntf, k128[:, 0:1], None, OP.is_lt)
            nc.vector.copy_predicated(hi, sel2, mid)

        # final masked sum + count at threshold lo
        st2 = small.tile([128, 2], F32, tag="st2")
        nc.vector.scalar_tensor_tensor(msk, lcp, lo[:, 0:1], lcp, OP.is_gt,
                                       OP.mult, accum_out=st2[:, 0:1])
        nc.vector.tensor_scalar(msk, lcp, lo[:, 0:1], None, OP.is_gt,
                                OP.add, accum_out=st2[:, 1:2])
        g2_ps = psum.tile([128, 2], F32, tag="ps")
        nc.tensor.matmul(g2_ps, G128, st2, start=True, stop=True)
        gt2 = small.tile([128, 2], F32, tag="gt2")
        nc.scalar.copy(gt2, g2_ps)
        sn = small.tile([128, 1], F32, tag="sn")
        nc.vector.tensor_tensor(sn, gt2[:, 1:2], k128, OP.subtract)
        nc.vector.tensor_tensor(sn, sn, lo, OP.mult)
        nc.vector.tensor_tensor(sn, gt2[:, 0:1], sn, OP.subtract)
        d_ps = psum.tile([1, 1], F32, tag="ps")
        nc.tensor.matmul(d_ps, sn, selone, start=True, stop=True)

        # final scalars A..E
        a_ps = psum.tile([1, 1], F32, tag="ps")
        nc.tensor.matmul(a_ps, llacc, ones_128x1, start=True, stop=True)
        b_ps = psum.tile([1, 1], F32, tag="ps")
        nc.tensor.matmul(b_ps, blacc, ones_128x1, start=True, stop=True)
        c2_ps = psum.tile([1, 1], F32, tag="ps")
        nc.tensor.matmul(c2_ps, bcacc, ones_128x1, start=True, stop=True)
        outsb = small.tile([1, 8], F32, tag="outsb")
        nc.vector.memset(outsb, 0.0)
        nc.scalar.copy(outsb[:, 0:1], a_ps)
        nc.scalar.copy(outsb[:, 1:2], b_ps)
        nc.scalar.copy(outsb[:, 2:3], c2_ps)
        nc.scalar.copy(outsb[:, 3:4], d_ps)
        nc.scalar.copy(outsb[:, 4:5], e_ps)
        nc.sync.dma_start(out=out, in_=outsb)


_CACHED = {}


def kernel(loc_data, conf_data, priors, targets):
    if "nc" not in _CACHED:
        _CACHED["nc"] = build_program()
    nc = _CACHED["nc"]
    in_maps = []
    for c in range(NCORES):
        sl = slice(c * R, (c + 1) * R)
        in_maps.append({
            "loc": np.ascontiguousarray(loc_data[sl]),
            "conf": np.ascontiguousarray(conf_data[sl]),
            "priors": np.ascontiguousarray(priors),
            "targets": np.ascontiguousarray(targets[sl]),
        })
    res = bass_utils.run_bass_kernel_spmd(nc, in_maps, core_ids=list(range(NCORES)))
    _CACHED["last_results"] = res
    A = Bs = Cs = D = E = 0.0
    for c in range(NCORES):
        o = res.results[c]["out"].reshape(-1)
        A += float(o[0]); Bs += float(o[1]); Cs += float(o[2])
        D += float(o[3]); E += float(o[4])
    N = max(E, 1.0)
    return np.array([A / N, (Bs - Cs + D) / N], dtype=np.float32)
